# revision 13
# baseline (speedup 1.0000x reference)
r"""Trainium2 Bass kernel for the CounterfactualODEModel problem.

Reference computes an adaptive dopri5 solve of
    dx/dt = MLP(concat(x, tr(t))),  tr = piecewise-linear treatments,
evaluated at the T=100 grid times.  Instead of replaying the sequential
solver on device, this kernel uses a parallel-in-time Picard iteration on
the integral form  x(t) = x0 + \int_0^t f(x(s), s) ds:

  - each sweep evaluates the MLP at a batch of sample times as batched
    matmuls and applies a cumulative-trapezoid quadrature matrix built on
    host from ts:  X <- x0 + A @ f(X).  The x0 / b3 constant terms ride
    two extra contraction rows of the integration matmul, so the updated
    state drops out of a single PSUM accumulation,
  - the iteration contracts ~10-25x per sweep; sweep 0 therefore runs on
    a 9x-coarsened time grid (12 samples, integrated back onto all 100
    grid times by a rectangular quadrature matrix) -- its extra coarse-
    grid error is wiped by the final full-grid sweep.  NSWEEP=2 lands at
    ~2.2e-3 relative error vs the f32 dopri5 reference (gate: 2e-2);
    within one treatment interval the integrand is smooth, so the
    trapezoid fixed point itself sits ~1.4e-4 away,
  - everything runs in bf16 (1 PE cycle/row at ANY moving-dim size,
    unlike f32r which needs >=256), so no free-dim padding is needed;
    PSUM accumulation stays fp32.

Implementation notes:
  - raw Bass (explicit engine streams + semaphores): the walrus build in
    this environment rejects instructions carrying more than one attached
    sync-wait, so standalone wait_ge instructions are used throughout,
  - inputs ride two parallel DMAs: the critical state+weights+biases
    chunk is issued by sync (the first engine to enter the block body),
    the quadrature chunk by gpsimd; a dep-free tanh on scratch data
    preloads the ACT table under the input DMAs,
  - the output DMA is issued against the LAST MATMUL's semaphore, not the
    fp32 PSUM->SBUF copy that feeds it: the issue instruction alone takes
    >=650ns while the DVE copy completes in ~410ns from the same trigger,
    so the copy always beats the DMA's first read -- this hides the copy
    entirely,
  - no memsets: every SBUF region a matmul reads is fully written either
    by a zero-padded host buffer or by a producer instruction.

The whole state is tiny (100 x 36), so the problem is replicated on all
8 cores (no useful tensor/batch parallelism exists for one trajectory);
core 0's output is returned.
"""

import numpy as np
import ml_dtypes

from contextlib import ExitStack

import concourse.bass as bass
import concourse.mybir as mybir
from concourse import bass_utils

T = 100
FD = 32   # feature dim
TD = 4    # treatment dim
HD = 64   # hidden dim
IN_DIM = FD + TD
STRIDE = 9               # coarse-grid stride for the non-final sweeps
NC = (T - 1) // STRIDE + 1   # 12 coarse samples (hits both endpoints)
NSWEEP = 2
PLAN = NSWEEP  # kept for test.py / prof.py compatibility
N_CORES = 8

_F32 = mybir.dt.float32
_BF = mybir.dt.bfloat16

# d ([102, DW] bf16) column layout:
#   STC | ST | W1 | B1 | B2 | W2 | W3 || ATC | ATA | FAB
_C_STC = 0                  # [36, NC] coarse init state (x0 bcast + tr[::9]^T)
_C_ST = _C_STC + NC         # [36, T]  full state (rows 32:36 = tr^T from host)
_C_W1 = _C_ST + T           # [36, 64]
_C_B1 = _C_W1 + HD          # fp32 [64,1] as 2 bf16 cols (byte off 4-aligned)
_C_B2 = _C_B1 + 2
_C_W2 = _C_B2 + 2           # [64, 64]
_C_W3 = _C_W2 + HD          # [64, 32]
_C1 = _C_W3 + FD            # end of DMA1 (rows 0:64)
_C_ATC = _C1                # [NC+2, T]: Ar^T | ones | ts-t0  (coarse->fine)
_C_ATA = _C_ATC + T         # [T+2, T]:  A^T  | ones | ts-t0  (fine->fine)
_C_FABC = _C_ATA + T        # [NC+2, 32]: rows NC:NC+2 = [x0; b3]; 0:NC live F
_C_FAB = _C_FABC + FD       # [T+2, 32]:  rows T:T+2   = [x0; b3]; 0:T  live F
DW = _C_FAB + FD
DP = T + 2                  # partition count


class _LeanBlock(bass.BassBlock):
    """Block whose exit skips the all-engine EVSEM butterfly: engines just
    drain and end.  Output integrity is guaranteed by the sync stream's
    final wait on the output-DMA semaphore; semaphores are re-cleared by
    the preamble on every execution."""

    def __exit__(self, exc_type, exc_val, exc_tb):
        if exc_type is not None:
            return
        for engine, last_body in self.last_body.items():
            with self.bass.body(
                last_body, parent=self.bass.cur_bb, allow_existing_parent=True
            ):
                engine.br(self.end_bb)
        self.bass.switch_bb(self.end_bb)
        sync_type = self.bass.sync.engine
        for eng_type, eng in self.bass.engines.items():
            if eng_type != sync_type:
                continue
            d = mybir.InstDrain(
                name=self.bass.get_next_instruction_name(),
                ins=[],
                outs=[],
                bass_is_fusable=False,
            )
            d.engine = eng_type
            eng.add_instruction(d)


def _build_nc(nsweep=NSWEEP):
    nc = bass.Bass(trn_type="TRN2", monotonic_sem_count=0, enable_partition_id=False)
    d = nc.dram_tensor("d", [DP, DW], _BF, kind="ExternalInput")
    xt = nc.dram_tensor("xt", [FD, T], _F32, kind="ExternalOutput")

    tanh = mybir.ActivationFunctionType.Tanh

    with ExitStack() as ctx:
        sb = lambda nm, shape, dt: ctx.enter_context(nc.sbuf_tensor(nm, shape, dt))
        ps = lambda nm, shape: ctx.enter_context(nc.psum_tensor(nm, shape, _F32))
        sem = lambda nm: ctx.enter_context(nc.semaphore(nm))
        t = sb("t_d", [DP, DW], _BF)
        h1 = sb("t_h1", [HD, T], _BF)
        h2 = sb("t_h2", [HD, T], _BF)
        warm = sb("t_warm", [HD, 1], _F32)
        xt_sb = sb("t_xt", [FD, T], _F32)
        p1 = ps("t_p1", [HD, T])
        p2 = ps("t_p2", [HD, T])
        pf = ps("t_pf", [T, FD])
        px = ps("t_px", [FD, T])
        sem_d1 = sem("sem_d1")
        sem_d2 = sem("sem_d2")
        c_sem = sem("sem_c")
        out_sem = sem("sem_out")

        st_c = t[0:IN_DIM, _C_STC:_C_STC + NC]
        st = t[0:IN_DIM, _C_ST:_C_ST + T]
        w1 = t[0:IN_DIM, _C_W1:_C_W1 + HD]
        w2 = t[0:HD, _C_W2:_C_W2 + HD]
        w3 = t[0:HD, _C_W3:_C_W3 + FD]
        b1 = t[0:HD, _C_B1:_C_B1 + 2].bitcast(_F32)
        b2 = t[0:HD, _C_B2:_C_B2 + 2].bitcast(_F32)
        atc = t[0:2 + NC, _C_ATC:_C_ATC + T]
        ata = t[0:DP, _C_ATA:_C_ATA + T]
        fab_c = t[0:2 + NC, _C_FABC:_C_FABC + FD]
        fab_f = t[0:DP, _C_FAB:_C_FAB + FD]

        block = ctx.enter_context(_LeanBlock(nc, 'blk'))

        # per-sweep sample count / operand views (last sweep = full grid)
        def sw(j):
            full = j == nsweep - 1
            n = T if full else NC
            return dict(
                n=n,
                stv=st if full else st_c,
                fab_w=t[0:n, (_C_FAB if full else _C_FABC):(_C_FAB if full else _C_FABC) + FD],
                mm4_l=fab_f if full else fab_c,
                mm4_r=ata if full else atc,
            )

        # semaphore plan (DMA sems inc by 16 at transfer completion):
        #   sem_d1: DMA1 (state+weights+biases) = 16
        #   sem_d2: DMA2 (quadrature + fab consts) = 16
        #   c_sem:  single chain semaphore -- the compute DAG is one strict
        #           serial chain (mm1,tanh1,mm2,tanh2,mm3,cast,mm4,copy) x
        #           nsweep, so every op incs c_sem and waits on its
        #           predecessor's count: op k of sweep j waits c >= 8j+k-1
        #   out_sem: output DMA = 16

        @block.gpsimd
        def _(gpsimd):
            nc.gpsimd.dma_start(t[0:DP, _C1:DW], d[0:DP, _C1:DW]).then_inc(sem_d2, 16)

        @block.sync
        def _(sync):
            sync.dma_start(t[0:HD, 0:_C1], d[0:HD, 0:_C1]).then_inc(sem_d1, 16)
            sync.wait_ge(c_sem, 8 * nsweep - 1)
            sync.dma_start(xt[:, :], xt_sb[:, :]).then_inc(out_sem, 16)
            sync.wait_ge(out_sem, 16)

        @block.tensor
        def _(tensor):
            tensor.wait_ge(sem_d1, 16)
            for j in range(nsweep):
                s = sw(j)
                n = s["n"]
                if j > 0:
                    tensor.wait_ge(c_sem, 8 * j)
                nc.tensor.matmul(p1[:, 0:n], w1, s["stv"], start=True, stop=True).then_inc(c_sem, 1)
                tensor.wait_ge(c_sem, 8 * j + 2)
                nc.tensor.matmul(p2[:, 0:n], w2, h1[:, 0:n], start=True, stop=True).then_inc(c_sem, 1)
                tensor.wait_ge(c_sem, 8 * j + 4)
                nc.tensor.matmul(pf[0:n, :], h2[:, 0:n], w3, start=True, stop=True).then_inc(c_sem, 1)
                tensor.wait_ge(c_sem, 8 * j + 6)
                if j == 0:
                    tensor.wait_ge(sem_d2, 16)
                nc.tensor.matmul(px[:, :], s["mm4_l"], s["mm4_r"], start=True, stop=True).then_inc(c_sem, 1)

        @block.scalar
        def _(scalar):
            # dep-free warm-up: zero a scratch tile, tanh it -> loads the
            # Tanh table while the input DMAs are still in flight.
            nc.scalar.mul(warm[:, :], warm[:, :], 0.0)
            nc.scalar.activation(warm[:, :], warm[:, :], tanh)
            for j in range(nsweep):
                n = sw(j)["n"]
                scalar.wait_ge(c_sem, 8 * j + 1)
                nc.scalar.activation(h1[:, 0:n], p1[:, 0:n], tanh, bias=b1).then_inc(c_sem, 1)
                scalar.wait_ge(c_sem, 8 * j + 3)
                nc.scalar.activation(h2[:, 0:n], p2[:, 0:n], tanh, bias=b2).then_inc(c_sem, 1)

        @block.vector
        def _(vector):
            for j in range(nsweep):
                s = sw(j)
                n = s["n"]
                vector.wait_ge(c_sem, 8 * j + 5)
                nc.vector.tensor_copy(s["fab_w"], pf[0:n, :]).then_inc(c_sem, 1)
                vector.wait_ge(c_sem, 8 * j + 7)
                if j < nsweep - 1:
                    nc.vector.tensor_copy(st[0:FD, :], px[:, :]).then_inc(c_sem, 1)
                else:
                    nc.vector.tensor_copy(xt_sb[:, :], px[:, :]).then_inc(c_sem, 1)

    return nc


_NC_CACHE = {}


def _get_nc(nsweep=NSWEEP):
    nsweep = int(nsweep)
    if nsweep not in _NC_CACHE:
        _NC_CACHE[nsweep] = _build_nc(nsweep)
    return _NC_CACHE[nsweep]


def _host_prep(x0, treatments, ts, W1, b1, W2, b2, W3, b3):
    ts64 = ts.astype(np.float64)
    x032 = np.ascontiguousarray(x0.reshape(FD), dtype=np.float32)

    # cumulative trapezoid quadrature matrix A [T,T]:
    # (A @ F)[t] ~= \int_{t_0}^{t_t} f dt  for F sampled at the grid times.
    h = np.diff(ts64)
    A = np.zeros((T, T), np.float64)
    for k in range(T - 1):
        A[k + 1] = A[k]
        A[k + 1, k] += h[k] / 2
        A[k + 1, k + 1] += h[k] / 2

    # rectangular coarse->fine matrix Ar [T, NC]: cumulative integral of the
    # piecewise-linear interpolant on the coarse grid, evaluated at fine ts.
    idx = np.arange(0, T, STRIDE)
    assert idx[-1] == T - 1 and len(idx) == NC
    tsc = ts64[idx]
    Ar = np.zeros((T, NC), np.float64)
    for ti in range(T):
        tt = ts64[ti]
        for k in range(NC - 1):
            t0c, t1c = tsc[k], tsc[k + 1]
            if tt <= t0c:
                break
            w = min(tt, t1c) - t0c
            frac = w / (t1c - t0c)
            Ar[ti, k] += w * (1 - frac / 2)
            Ar[ti, k + 1] += w * frac / 2

    D = np.zeros((DP, DW), dtype=ml_dtypes.bfloat16)
    D[0:FD, _C_STC:_C_STC + NC] = x032[:, None]
    D[FD:IN_DIM, _C_STC:_C_STC + NC] = treatments[idx].T
    D[FD:IN_DIM, _C_ST:_C_ST + T] = treatments.T
    D[0:IN_DIM, _C_W1:_C_W1 + HD] = W1
    D[0:HD, _C_W2:_C_W2 + HD] = W2
    D[0:HD, _C_W3:_C_W3 + FD] = W3
    for off, foff, M in ((_C_ATC, _C_FABC, Ar), (_C_ATA, _C_FAB, A)):
        ncol = M.shape[1]
        D[0:ncol, off:off + T] = M.T
        D[ncol, off:off + T] = 1.0
        D[ncol + 1, off:off + T] = ts64 - ts64[0]
        D[ncol, foff:foff + FD] = x032
        D[ncol + 1, foff:foff + FD] = b3
    # biases stay exact fp32, stored as raw bf16-bit pairs
    u16 = D.view(np.uint16)
    u16[0:HD, _C_B1:_C_B1 + 2] = np.ascontiguousarray(
        b1.reshape(HD, 1), dtype=np.float32).view(np.uint16)
    u16[0:HD, _C_B2:_C_B2 + 2] = np.ascontiguousarray(
        b2.reshape(HD, 1), dtype=np.float32).view(np.uint16)
    return {"d": D}


def kernel(x0, treatments, ts, W1, b1, W2, b2, W3, b3, _results=None, _plan=NSWEEP):
    in_map = _host_prep(x0, treatments, ts, W1, b1, W2, b2, W3, b3)
    nc = _get_nc(_plan)
    res = bass_utils.run_bass_kernel_spmd(
        nc, [in_map] * N_CORES, core_ids=list(range(N_CORES))
    )
    if _results is not None:
        _results.append(res)
    xt = res.results[0]["xt"]  # [FD, T]
    out = xt.T.reshape(T, 1, FD)
    return np.ascontiguousarray(out, dtype=np.float32)


# revision 14
# speedup vs baseline: 1.2659x; 1.2659x over previous
r"""Trainium2 Bass kernel for the CounterfactualODEModel problem.

Reference computes an adaptive dopri5 solve of
    dx/dt = MLP(concat(x, tr(t))),  tr = piecewise-linear treatments,
evaluated at the T=100 grid times.  Instead of replaying the sequential
solver on device, this kernel uses a parallel-in-time Picard iteration on
the integral form  x(t) = x0 + \int_0^t f(x(s), s) ds:

  - each sweep evaluates the MLP at a batch of sample times as batched
    matmuls and applies a cumulative-trapezoid quadrature matrix built on
    host from ts:  X <- x0 + A @ f(X).  The x0 / b3 constant terms ride
    two extra contraction rows of the integration matmul, so the updated
    state drops out of a single PSUM accumulation,
  - the iteration contracts ~10-25x per sweep; sweep 0 therefore runs on
    a 9x-coarsened time grid (12 samples, integrated back onto all 100
    grid times by a rectangular quadrature matrix) -- its extra coarse-
    grid error is wiped by the final full-grid sweep.  NSWEEP=2 lands at
    ~2.2e-3 relative error vs the f32 dopri5 reference (gate: 2e-2);
    within one treatment interval the integrand is smooth, so the
    trapezoid fixed point itself sits ~1.4e-4 away,
  - everything runs in bf16 (1 PE cycle/row at ANY moving-dim size,
    unlike f32r which needs >=256), so no free-dim padding is needed;
    PSUM accumulation stays fp32.

Implementation notes:
  - raw Bass (explicit engine streams + semaphores): the walrus build in
    this environment rejects instructions carrying more than one attached
    sync-wait, so standalone wait_ge instructions are used throughout,
  - inputs ride two parallel DMAs: the critical state+weights+biases
    chunk is issued by sync (the first engine to enter the block body),
    the quadrature chunk by gpsimd; a dep-free tanh on scratch data
    preloads the ACT table under the input DMAs,
  - the output DMA is issued against the LAST MATMUL's semaphore, not the
    fp32 PSUM->SBUF copy that feeds it: the issue instruction alone takes
    >=650ns while the DVE copy completes in ~410ns from the same trigger,
    so the copy always beats the DMA's first read -- this hides the copy
    entirely,
  - no memsets: every SBUF region a matmul reads is fully written either
    by a zero-padded host buffer or by a producer instruction.

The whole state is tiny (100 x 36), so the problem is replicated on all
8 cores (no useful tensor/batch parallelism exists for one trajectory);
core 0's output is returned.
"""

import numpy as np
import ml_dtypes

from contextlib import ExitStack

import concourse.bass as bass
import concourse.mybir as mybir
from concourse import bass_utils

T = 100
FD = 32   # feature dim
TD = 4    # treatment dim
HD = 64   # hidden dim
IN_DIM = FD + TD
STRIDE = 9               # coarse-grid stride for the non-final sweeps
NC = (T - 1) // STRIDE + 1   # 12 coarse samples (hits both endpoints)
NSWEEP = 2
PLAN = NSWEEP  # kept for test.py / prof.py compatibility
N_CORES = 8

_F32 = mybir.dt.float32
_BF = mybir.dt.bfloat16

# d ([102, DW] bf16) column layout:
#   STC | ST | W1 | B1 | B2 | W2 | W3 || ATC | ATA | FAB
_C_STC = 0                  # [36, NC] coarse init state (x0 bcast + tr[::9]^T)
_C_ST = _C_STC + NC         # [36, T]  full state (rows 32:36 = tr^T from host)
_C_W1 = _C_ST + T           # [36, 64]
_C_B1 = _C_W1 + HD          # fp32 [64,1] as 2 bf16 cols (byte off 4-aligned)
_C_B2 = _C_B1 + 2
_C_W2 = _C_B2 + 2           # [64, 64]
_C_W3 = _C_W2 + HD          # [64, 32]
_C1 = _C_W3 + FD            # end of DMA1 (rows 0:64)
_C_ATC = _C1                # [NC+2, T]: Ar^T | ones | ts-t0  (coarse->fine)
_C_ATA = _C_ATC + T         # [T+2, T]:  A^T  | ones | ts-t0  (fine->fine)
_C_FABC = _C_ATA + T        # [NC+2, 32]: rows NC:NC+2 = [x0; b3]; 0:NC live F
_C_FAB = _C_FABC + FD       # [T+2, 32]:  rows T:T+2   = [x0; b3]; 0:T  live F
DW = _C_FAB + FD
DP = T + 2                  # partition count


class _LeanBlock(bass.BassBlock):
    """Block whose exit skips the all-engine EVSEM butterfly: engines just
    drain and end.  Output integrity is guaranteed by the sync stream's
    final wait on the output-DMA semaphore; semaphores are re-cleared by
    the preamble on every execution."""

    def __exit__(self, exc_type, exc_val, exc_tb):
        if exc_type is not None:
            return
        for engine, last_body in self.last_body.items():
            with self.bass.body(
                last_body, parent=self.bass.cur_bb, allow_existing_parent=True
            ):
                engine.br(self.end_bb)
        self.bass.switch_bb(self.end_bb)
        sync_type = self.bass.sync.engine
        for eng_type, eng in self.bass.engines.items():
            if eng_type != sync_type:
                continue
            d = mybir.InstDrain(
                name=self.bass.get_next_instruction_name(),
                ins=[],
                outs=[],
                bass_is_fusable=False,
            )
            d.engine = eng_type
            eng.add_instruction(d)


def _build_nc(nsweep=NSWEEP):
    nc = bass.Bass(trn_type="TRN2", monotonic_sem_count=0, enable_partition_id=False)
    d = nc.dram_tensor("d", [DP, DW], _BF, kind="ExternalInput")
    xt = nc.dram_tensor("xt", [FD, T], _F32, kind="ExternalOutput")

    tanh = mybir.ActivationFunctionType.Tanh

    with ExitStack() as ctx:
        sb = lambda nm, shape, dt: ctx.enter_context(nc.sbuf_tensor(nm, shape, dt))
        ps = lambda nm, shape: ctx.enter_context(nc.psum_tensor(nm, shape, _F32))
        sem = lambda nm: ctx.enter_context(nc.semaphore(nm))
        t = sb("t_d", [DP, DW], _BF)
        h1 = sb("t_h1", [HD, T], _BF)
        h2 = sb("t_h2", [HD, T], _BF)
        warm = sb("t_warm", [HD, 1], _F32)
        xt_sb = sb("t_xt", [FD, T], _F32)
        p1 = ps("t_p1", [HD, T])
        p2 = ps("t_p2", [HD, T])
        pf = ps("t_pf", [T, FD])
        px = ps("t_px", [FD, T])
        sem_d1 = sem("sem_d1")
        sem_d2 = sem("sem_d2")
        pe_sem = sem("sem_pe")
        act_sem = sem("sem_act")
        dve_sem = sem("sem_dve")
        out_sem = sem("sem_out")

        st_c = t[0:IN_DIM, _C_STC:_C_STC + NC]
        st = t[0:IN_DIM, _C_ST:_C_ST + T]
        w1 = t[0:IN_DIM, _C_W1:_C_W1 + HD]
        w2 = t[0:HD, _C_W2:_C_W2 + HD]
        w3 = t[0:HD, _C_W3:_C_W3 + FD]
        b1 = t[0:HD, _C_B1:_C_B1 + 2].bitcast(_F32)
        b2 = t[0:HD, _C_B2:_C_B2 + 2].bitcast(_F32)
        atc = t[0:2 + NC, _C_ATC:_C_ATC + T]
        ata = t[0:DP, _C_ATA:_C_ATA + T]
        fab_c = t[0:2 + NC, _C_FABC:_C_FABC + FD]
        fab_f = t[0:DP, _C_FAB:_C_FAB + FD]

        block = ctx.enter_context(_LeanBlock(nc, 'blk'))

        # per-sweep sample count / operand views (last sweep = full grid)
        def sw(j):
            full = j == nsweep - 1
            n = T if full else NC
            return dict(
                n=n,
                stv=st if full else st_c,
                fab_w=t[0:n, (_C_FAB if full else _C_FABC):(_C_FAB if full else _C_FABC) + FD],
                mm4_l=fab_f if full else fab_c,
                mm4_r=ata if full else atc,
            )

        # semaphore plan (DMA sems inc by 16 at transfer completion):
        #   sem_d1: DMA1 (state+weights+biases) = 16
        #   sem_d2: DMA2 (quadrature + fab consts) = 16
        #   pe_sem:  4 matmuls/sweep -> 4j+k after k-th matmul of sweep j
        #   act_sem: 2 tanhs/sweep   -> 2j+k
        #   dve_sem: 2 copies/sweep (fab, state; last sweep: fab, fp32 out)
        #   out_sem: output DMA = 16

        @block.gpsimd
        def _(gpsimd):
            nc.gpsimd.dma_start(t[0:DP, _C1:DW], d[0:DP, _C1:DW]).then_inc(sem_d2, 16)

        @block.sync
        def _(sync):
            sync.dma_start(t[0:HD, 0:_C1], d[0:HD, 0:_C1]).then_inc(sem_d1, 16)
            sync.wait_ge(pe_sem, 4 * nsweep)
            sync.dma_start(xt[:, :], xt_sb[:, :]).then_inc(out_sem, 16)
            sync.wait_ge(out_sem, 16)

        @block.tensor
        def _(tensor):
            tensor.wait_ge(sem_d1, 16)
            for j in range(nsweep):
                s = sw(j)
                n = s["n"]
                if j > 0:
                    tensor.wait_ge(dve_sem, 2 * j)
                nc.tensor.matmul(p1[:, 0:n], w1, s["stv"], start=True, stop=True).then_inc(pe_sem, 1)
                tensor.wait_ge(act_sem, 2 * j + 1)
                nc.tensor.matmul(p2[:, 0:n], w2, h1[:, 0:n], start=True, stop=True).then_inc(pe_sem, 1)
                tensor.wait_ge(act_sem, 2 * j + 2)
                nc.tensor.matmul(pf[0:n, :], h2[:, 0:n], w3, start=True, stop=True).then_inc(pe_sem, 1)
                tensor.wait_ge(dve_sem, 2 * j + 1)
                if j == 0:
                    tensor.wait_ge(sem_d2, 16)
                nc.tensor.matmul(px[:, :], s["mm4_l"], s["mm4_r"], start=True, stop=True).then_inc(pe_sem, 1)

        @block.scalar
        def _(scalar):
            # dep-free warm-up: zero a scratch tile, tanh it -> loads the
            # Tanh table while the input DMAs are still in flight.
            nc.scalar.mul(warm[:, :], warm[:, :], 0.0)
            nc.scalar.activation(warm[:, :], warm[:, :], tanh)
            for j in range(nsweep):
                n = sw(j)["n"]
                scalar.wait_ge(pe_sem, 4 * j + 1)
                nc.scalar.activation(h1[:, 0:n], p1[:, 0:n], tanh, bias=b1).then_inc(act_sem, 1)
                scalar.wait_ge(pe_sem, 4 * j + 2)
                nc.scalar.activation(h2[:, 0:n], p2[:, 0:n], tanh, bias=b2).then_inc(act_sem, 1)

        @block.vector
        def _(vector):
            for j in range(nsweep):
                s = sw(j)
                n = s["n"]
                vector.wait_ge(pe_sem, 4 * j + 3)
                nc.vector.tensor_copy(s["fab_w"], pf[0:n, :]).then_inc(dve_sem, 1)
                vector.wait_ge(pe_sem, 4 * j + 4)
                if j < nsweep - 1:
                    nc.vector.tensor_copy(st[0:FD, :], px[:, :]).then_inc(dve_sem, 1)
                else:
                    nc.vector.tensor_copy(xt_sb[:, :], px[:, :]).then_inc(dve_sem, 1)

    return nc


_NC_CACHE = {}


def _get_nc(nsweep=NSWEEP):
    nsweep = int(nsweep)
    if nsweep not in _NC_CACHE:
        _NC_CACHE[nsweep] = _build_nc(nsweep)
    return _NC_CACHE[nsweep]


def _host_prep(x0, treatments, ts, W1, b1, W2, b2, W3, b3):
    ts64 = ts.astype(np.float64)
    x032 = np.ascontiguousarray(x0.reshape(FD), dtype=np.float32)

    # cumulative trapezoid quadrature matrix A [T,T]:
    # (A @ F)[t] ~= \int_{t_0}^{t_t} f dt  for F sampled at the grid times.
    h = np.diff(ts64)
    A = np.zeros((T, T), np.float64)
    for k in range(T - 1):
        A[k + 1] = A[k]
        A[k + 1, k] += h[k] / 2
        A[k + 1, k + 1] += h[k] / 2

    # rectangular coarse->fine matrix Ar [T, NC]: cumulative integral of the
    # piecewise-linear interpolant on the coarse grid, evaluated at fine ts.
    idx = np.arange(0, T, STRIDE)
    assert idx[-1] == T - 1 and len(idx) == NC
    tsc = ts64[idx]
    Ar = np.zeros((T, NC), np.float64)
    for ti in range(T):
        tt = ts64[ti]
        for k in range(NC - 1):
            t0c, t1c = tsc[k], tsc[k + 1]
            if tt <= t0c:
                break
            w = min(tt, t1c) - t0c
            frac = w / (t1c - t0c)
            Ar[ti, k] += w * (1 - frac / 2)
            Ar[ti, k + 1] += w * frac / 2

    D = np.zeros((DP, DW), dtype=ml_dtypes.bfloat16)
    D[0:FD, _C_STC:_C_STC + NC] = x032[:, None]
    D[FD:IN_DIM, _C_STC:_C_STC + NC] = treatments[idx].T
    D[FD:IN_DIM, _C_ST:_C_ST + T] = treatments.T
    D[0:IN_DIM, _C_W1:_C_W1 + HD] = W1
    D[0:HD, _C_W2:_C_W2 + HD] = W2
    D[0:HD, _C_W3:_C_W3 + FD] = W3
    for off, foff, M in ((_C_ATC, _C_FABC, Ar), (_C_ATA, _C_FAB, A)):
        ncol = M.shape[1]
        D[0:ncol, off:off + T] = M.T
        D[ncol, off:off + T] = 1.0
        D[ncol + 1, off:off + T] = ts64 - ts64[0]
        D[ncol, foff:foff + FD] = x032
        D[ncol + 1, foff:foff + FD] = b3
    # biases stay exact fp32, stored as raw bf16-bit pairs
    u16 = D.view(np.uint16)
    u16[0:HD, _C_B1:_C_B1 + 2] = np.ascontiguousarray(
        b1.reshape(HD, 1), dtype=np.float32).view(np.uint16)
    u16[0:HD, _C_B2:_C_B2 + 2] = np.ascontiguousarray(
        b2.reshape(HD, 1), dtype=np.float32).view(np.uint16)
    return {"d": D}


def kernel(x0, treatments, ts, W1, b1, W2, b2, W3, b3, _results=None, _plan=NSWEEP):
    in_map = _host_prep(x0, treatments, ts, W1, b1, W2, b2, W3, b3)
    nc = _get_nc(_plan)
    res = bass_utils.run_bass_kernel_spmd(
        nc, [in_map] * N_CORES, core_ids=list(range(N_CORES))
    )
    if _results is not None:
        _results.append(res)
    xt = res.results[0]["xt"]  # [FD, T]
    out = xt.T.reshape(T, 1, FD)
    return np.ascontiguousarray(out, dtype=np.float32)


# revision 15
# speedup vs baseline: 1.3071x; 1.0325x over previous
r"""Trainium2 Bass kernel for the CounterfactualODEModel problem.

Reference computes an adaptive dopri5 solve of
    dx/dt = MLP(concat(x, tr(t))),  tr = piecewise-linear treatments,
evaluated at the T=100 grid times.  Instead of replaying the sequential
solver on device, this kernel uses a parallel-in-time Picard iteration on
the integral form  x(t) = x0 + \int_0^t f(x(s), s) ds:

  - each sweep evaluates the MLP at a batch of sample times as batched
    matmuls and applies a cumulative-trapezoid quadrature matrix built on
    host from ts:  X <- x0 + A @ f(X).  The x0 / b3 constant terms ride
    two extra contraction rows of the integration matmul, so the updated
    state drops out of a single PSUM accumulation,
  - the iteration contracts ~10-25x per sweep; sweep 0 therefore runs on
    a 9x-coarsened time grid (12 samples, integrated back onto all 100
    grid times by a rectangular quadrature matrix) -- its extra coarse-
    grid error is wiped by the final full-grid sweep.  NSWEEP=2 lands at
    ~2.2e-3 relative error vs the f32 dopri5 reference (gate: 2e-2);
    within one treatment interval the integrand is smooth, so the
    trapezoid fixed point itself sits ~1.4e-4 away,
  - everything runs in bf16 (1 PE cycle/row at ANY moving-dim size,
    unlike f32r which needs >=256), so no free-dim padding is needed;
    PSUM accumulation stays fp32.

Implementation notes:
  - raw Bass (explicit engine streams + semaphores): the walrus build in
    this environment rejects instructions carrying more than one attached
    sync-wait, so standalone wait_ge instructions are used throughout,
  - inputs ride two parallel DMAs: the critical state+weights+biases
    chunk is issued by sync (the first engine to enter the block body),
    the quadrature chunk by gpsimd; a dep-free tanh on scratch data
    preloads the ACT table under the input DMAs,
  - the output DMA is issued against the LAST MATMUL's semaphore, not the
    fp32 PSUM->SBUF copy that feeds it: the issue instruction alone takes
    >=650ns while the DVE copy completes in ~410ns from the same trigger,
    so the copy always beats the DMA's first read -- this hides the copy
    entirely,
  - no memsets: every SBUF region a matmul reads is fully written either
    by a zero-padded host buffer or by a producer instruction.

The whole state is tiny (100 x 36), so the problem is replicated on all
8 cores (no useful tensor/batch parallelism exists for one trajectory);
core 0's output is returned.
"""

import numpy as np
import ml_dtypes

from contextlib import ExitStack

import concourse.bass as bass
import concourse.mybir as mybir
from concourse import bass_utils

T = 100
FD = 32   # feature dim
TD = 4    # treatment dim
HD = 64   # hidden dim
IN_DIM = FD + TD
STRIDE = 9               # coarse-grid stride for the non-final sweeps
NC = (T - 1) // STRIDE + 1   # 12 coarse samples (hits both endpoints)
NSWEEP = 2
PLAN = NSWEEP  # kept for test.py / prof.py compatibility
N_CORES = 8

_F32 = mybir.dt.float32
_BF = mybir.dt.bfloat16

# d ([102, DW] bf16) column layout:
#   STC | ST | W1 | B1 | B2 | W2 | W3 || ATC | ATA | FAB
_C_STC = 0                  # [36, NC] coarse init state (x0 bcast + tr[::9]^T)
_C_ST = _C_STC + NC         # [36, T]  full state (rows 32:36 = tr^T from host)
_C_W1 = _C_ST + T           # [36, 64]
_C_B1 = _C_W1 + HD          # fp32 [64,1] as 2 bf16 cols (byte off 4-aligned)
_C_B2 = _C_B1 + 2
_C_W2 = _C_B2 + 2           # [64, 64]
_C_W3 = _C_W2 + HD          # [64, 32]
_C1 = _C_W3 + FD            # end of DMA1 (rows 0:64)
_C_ATC = _C1                # [NC+2, T]: Ar^T | ones | ts-t0  (coarse->fine)
_C_ATA = _C_ATC + T         # [T+2, T]:  A^T  | ones | ts-t0  (fine->fine)
_C_FABC = _C_ATA + T        # [NC+2, 32]: rows NC:NC+2 = [x0; b3]; 0:NC live F
_C_FAB = _C_FABC + FD       # [T+2, 32]:  rows T:T+2   = [x0; b3]; 0:T  live F
DW = _C_FAB + FD
DP = T + 2                  # partition count


class _LeanBlock(bass.BassBlock):
    """Block whose exit skips the all-engine EVSEM butterfly: engines just
    drain and end.  Output integrity is guaranteed by the sync stream's
    final wait on the output-DMA semaphore; semaphores are re-cleared by
    the preamble on every execution."""

    def __exit__(self, exc_type, exc_val, exc_tb):
        if exc_type is not None:
            return
        for engine, last_body in self.last_body.items():
            with self.bass.body(
                last_body, parent=self.bass.cur_bb, allow_existing_parent=True
            ):
                engine.br(self.end_bb)
        self.bass.switch_bb(self.end_bb)
        sync_type = self.bass.sync.engine
        for eng_type, eng in self.bass.engines.items():
            if eng_type != sync_type:
                continue
            d = mybir.InstDrain(
                name=self.bass.get_next_instruction_name(),
                ins=[],
                outs=[],
                bass_is_fusable=False,
            )
            d.engine = eng_type
            eng.add_instruction(d)


def _build_nc(nsweep=NSWEEP):
    nc = bass.Bass(trn_type="TRN2", monotonic_sem_count=0, enable_partition_id=False)
    d = nc.dram_tensor("d", [DP, DW], _BF, kind="ExternalInput")
    xt = nc.dram_tensor("xt", [FD, T], _F32, kind="ExternalOutput")

    tanh = mybir.ActivationFunctionType.Tanh

    with ExitStack() as ctx:
        sb = lambda nm, shape, dt: ctx.enter_context(nc.sbuf_tensor(nm, shape, dt))
        ps = lambda nm, shape: ctx.enter_context(nc.psum_tensor(nm, shape, _F32))
        sem = lambda nm: ctx.enter_context(nc.semaphore(nm))
        t = sb("t_d", [DP, DW], _BF)
        h1 = sb("t_h1", [HD, T], _BF)
        h2 = sb("t_h2", [HD, T], _BF)
        warm = sb("t_warm", [HD, 1], _F32)
        xt_sb = sb("t_xt", [FD, T], _F32)
        p1 = ps("t_p1", [HD, T])
        p2 = ps("t_p2", [HD, T])
        pf = ps("t_pf", [T, FD])
        px = ps("t_px", [FD, T])
        sem_d1 = sem("sem_d1")
        sem_d2 = sem("sem_d2")
        pe_sem = sem("sem_pe")
        act_sem = sem("sem_act")
        dve_sem = sem("sem_dve")

        st_c = t[0:IN_DIM, _C_STC:_C_STC + NC]
        st = t[0:IN_DIM, _C_ST:_C_ST + T]
        w1 = t[0:IN_DIM, _C_W1:_C_W1 + HD]
        w2 = t[0:HD, _C_W2:_C_W2 + HD]
        w3 = t[0:HD, _C_W3:_C_W3 + FD]
        b1 = t[0:HD, _C_B1:_C_B1 + 2].bitcast(_F32)
        b2 = t[0:HD, _C_B2:_C_B2 + 2].bitcast(_F32)
        atc = t[0:2 + NC, _C_ATC:_C_ATC + T]
        ata = t[0:DP, _C_ATA:_C_ATA + T]
        fab_c = t[0:2 + NC, _C_FABC:_C_FABC + FD]
        fab_f = t[0:DP, _C_FAB:_C_FAB + FD]

        block = ctx.enter_context(_LeanBlock(nc, 'blk'))

        # per-sweep sample count / operand views (last sweep = full grid)
        def sw(j):
            full = j == nsweep - 1
            n = T if full else NC
            return dict(
                n=n,
                stv=st if full else st_c,
                fab_w=t[0:n, (_C_FAB if full else _C_FABC):(_C_FAB if full else _C_FABC) + FD],
                mm4_l=fab_f if full else fab_c,
                mm4_r=ata if full else atc,
            )

        # semaphore plan (DMA sems inc by 16 at transfer completion):
        #   sem_d1: DMA1 (state+weights+biases) = 16; output DMA -> 32
        #   sem_d2: DMA2 (quadrature + fab consts) = 16
        #   pe_sem:  4 matmuls/sweep -> 4j+k after k-th matmul of sweep j
        #   act_sem: 2 tanhs/sweep   -> 2j+k
        #   dve_sem: 2 copies/sweep (fab, state; last sweep: fab, fp32 out)
        #   (output DMA completion rides sem_d1 -> 32)

        @block.gpsimd
        def _(gpsimd):
            nc.gpsimd.dma_start(t[0:DP, _C1:DW], d[0:DP, _C1:DW]).then_inc(sem_d2, 16)

        @block.sync
        def _(sync):
            sync.dma_start(t[0:HD, 0:_C1], d[0:HD, 0:_C1]).then_inc(sem_d1, 16)
            sync.wait_ge(pe_sem, 4 * nsweep)
            sync.dma_start(xt[:, :], xt_sb[:, :]).then_inc(sem_d1, 16)
            sync.wait_ge(sem_d1, 32)

        @block.tensor
        def _(tensor):
            tensor.wait_ge(sem_d1, 16)
            for j in range(nsweep):
                s = sw(j)
                n = s["n"]
                if j > 0:
                    tensor.wait_ge(dve_sem, 2 * j)
                nc.tensor.matmul(p1[:, 0:n], w1, s["stv"], start=True, stop=True).then_inc(pe_sem, 1)
                tensor.wait_ge(act_sem, 2 * j + 1)
                nc.tensor.matmul(p2[:, 0:n], w2, h1[:, 0:n], start=True, stop=True).then_inc(pe_sem, 1)
                tensor.wait_ge(act_sem, 2 * j + 2)
                nc.tensor.matmul(pf[0:n, :], h2[:, 0:n], w3, start=True, stop=True).then_inc(pe_sem, 1)
                tensor.wait_ge(dve_sem, 2 * j + 1)
                if j == 0:
                    tensor.wait_ge(sem_d2, 16)
                nc.tensor.matmul(px[:, :], s["mm4_l"], s["mm4_r"], start=True, stop=True).then_inc(pe_sem, 1)

        @block.scalar
        def _(scalar):
            # dep-free warm-up: zero a scratch tile, tanh it -> loads the
            # Tanh table while the input DMAs are still in flight.
            nc.scalar.mul(warm[:, :], warm[:, :], 0.0)
            nc.scalar.activation(warm[:, :], warm[:, :], tanh)
            for j in range(nsweep):
                n = sw(j)["n"]
                scalar.wait_ge(pe_sem, 4 * j + 1)
                nc.scalar.activation(h1[:, 0:n], p1[:, 0:n], tanh, bias=b1).then_inc(act_sem, 1)
                scalar.wait_ge(pe_sem, 4 * j + 2)
                nc.scalar.activation(h2[:, 0:n], p2[:, 0:n], tanh, bias=b2).then_inc(act_sem, 1)

        @block.vector
        def _(vector):
            for j in range(nsweep):
                s = sw(j)
                n = s["n"]
                vector.wait_ge(pe_sem, 4 * j + 3)
                nc.vector.tensor_copy(s["fab_w"], pf[0:n, :]).then_inc(dve_sem, 1)
                vector.wait_ge(pe_sem, 4 * j + 4)
                if j < nsweep - 1:
                    nc.vector.tensor_copy(st[0:FD, :], px[:, :]).then_inc(dve_sem, 1)
                else:
                    nc.vector.tensor_copy(xt_sb[:, :], px[:, :]).then_inc(dve_sem, 1)

    return nc


_NC_CACHE = {}


def _get_nc(nsweep=NSWEEP):
    nsweep = int(nsweep)
    if nsweep not in _NC_CACHE:
        _NC_CACHE[nsweep] = _build_nc(nsweep)
    return _NC_CACHE[nsweep]


def _host_prep(x0, treatments, ts, W1, b1, W2, b2, W3, b3):
    ts64 = ts.astype(np.float64)
    x032 = np.ascontiguousarray(x0.reshape(FD), dtype=np.float32)

    # cumulative trapezoid quadrature matrix A [T,T]:
    # (A @ F)[t] ~= \int_{t_0}^{t_t} f dt  for F sampled at the grid times.
    h = np.diff(ts64)
    A = np.zeros((T, T), np.float64)
    for k in range(T - 1):
        A[k + 1] = A[k]
        A[k + 1, k] += h[k] / 2
        A[k + 1, k + 1] += h[k] / 2

    # rectangular coarse->fine matrix Ar [T, NC]: cumulative integral of the
    # piecewise-linear interpolant on the coarse grid, evaluated at fine ts.
    idx = np.arange(0, T, STRIDE)
    assert idx[-1] == T - 1 and len(idx) == NC
    tsc = ts64[idx]
    Ar = np.zeros((T, NC), np.float64)
    for ti in range(T):
        tt = ts64[ti]
        for k in range(NC - 1):
            t0c, t1c = tsc[k], tsc[k + 1]
            if tt <= t0c:
                break
            w = min(tt, t1c) - t0c
            frac = w / (t1c - t0c)
            Ar[ti, k] += w * (1 - frac / 2)
            Ar[ti, k + 1] += w * frac / 2

    D = np.zeros((DP, DW), dtype=ml_dtypes.bfloat16)
    D[0:FD, _C_STC:_C_STC + NC] = x032[:, None]
    D[FD:IN_DIM, _C_STC:_C_STC + NC] = treatments[idx].T
    D[FD:IN_DIM, _C_ST:_C_ST + T] = treatments.T
    D[0:IN_DIM, _C_W1:_C_W1 + HD] = W1
    D[0:HD, _C_W2:_C_W2 + HD] = W2
    D[0:HD, _C_W3:_C_W3 + FD] = W3
    for off, foff, M in ((_C_ATC, _C_FABC, Ar), (_C_ATA, _C_FAB, A)):
        ncol = M.shape[1]
        D[0:ncol, off:off + T] = M.T
        D[ncol, off:off + T] = 1.0
        D[ncol + 1, off:off + T] = ts64 - ts64[0]
        D[ncol, foff:foff + FD] = x032
        D[ncol + 1, foff:foff + FD] = b3
    # biases stay exact fp32, stored as raw bf16-bit pairs
    u16 = D.view(np.uint16)
    u16[0:HD, _C_B1:_C_B1 + 2] = np.ascontiguousarray(
        b1.reshape(HD, 1), dtype=np.float32).view(np.uint16)
    u16[0:HD, _C_B2:_C_B2 + 2] = np.ascontiguousarray(
        b2.reshape(HD, 1), dtype=np.float32).view(np.uint16)
    return {"d": D}


def kernel(x0, treatments, ts, W1, b1, W2, b2, W3, b3, _results=None, _plan=NSWEEP):
    in_map = _host_prep(x0, treatments, ts, W1, b1, W2, b2, W3, b3)
    nc = _get_nc(_plan)
    res = bass_utils.run_bass_kernel_spmd(
        nc, [in_map] * N_CORES, core_ids=list(range(N_CORES))
    )
    if _results is not None:
        _results.append(res)
    xt = res.results[0]["xt"]  # [FD, T]
    out = xt.T.reshape(T, 1, FD)
    return np.ascontiguousarray(out, dtype=np.float32)


# revision 16
# speedup vs baseline: 1.3092x; 1.0016x over previous
r"""Trainium2 Bass kernel for the CounterfactualODEModel problem.

Reference computes an adaptive dopri5 solve of
    dx/dt = MLP(concat(x, tr(t))),  tr = piecewise-linear treatments,
evaluated at the T=100 grid times.  Instead of replaying the sequential
solver on device, this kernel uses a parallel-in-time Picard iteration on
the integral form  x(t) = x0 + \int_0^t f(x(s), s) ds:

  - each sweep evaluates the MLP at a batch of sample times as batched
    matmuls and applies a cumulative-trapezoid quadrature matrix built on
    host from ts:  X <- x0 + A @ f(X).  The x0 / b3 constant terms ride
    two extra contraction rows of the integration matmul, so the updated
    state drops out of a single PSUM accumulation,
  - the iteration contracts ~10-25x per sweep; sweep 0 therefore runs on
    a 9x-coarsened time grid (12 samples, integrated back onto all 100
    grid times by a rectangular quadrature matrix) -- its extra coarse-
    grid error is wiped by the final full-grid sweep.  NSWEEP=2 lands at
    ~2.2e-3 relative error vs the f32 dopri5 reference (gate: 2e-2);
    within one treatment interval the integrand is smooth, so the
    trapezoid fixed point itself sits ~1.4e-4 away,
  - everything runs in bf16 (1 PE cycle/row at ANY moving-dim size,
    unlike f32r which needs >=256), so no free-dim padding is needed;
    PSUM accumulation stays fp32.

Implementation notes:
  - raw Bass (explicit engine streams + semaphores): the walrus build in
    this environment rejects instructions carrying more than one attached
    sync-wait, so standalone wait_ge instructions are used throughout,
  - inputs ride two parallel DMAs: the critical state+weights+biases
    chunk is issued by sync (the first engine to enter the block body),
    the quadrature chunk by gpsimd; a dep-free tanh on scratch data
    preloads the ACT table under the input DMAs,
  - the output DMA is issued against the LAST MATMUL's semaphore, not the
    fp32 PSUM->SBUF copy that feeds it: the issue instruction alone takes
    >=650ns while the DVE copy completes in ~410ns from the same trigger,
    so the copy always beats the DMA's first read -- this hides the copy
    entirely,
  - no memsets: every SBUF region a matmul reads is fully written either
    by a zero-padded host buffer or by a producer instruction.

The whole state is tiny (100 x 36), so the problem is replicated on all
8 cores (no useful tensor/batch parallelism exists for one trajectory);
core 0's output is returned.
"""

import numpy as np
import ml_dtypes

from contextlib import ExitStack

import concourse.bass as bass
import concourse.mybir as mybir
from concourse import bass_utils

T = 100
FD = 32   # feature dim
TD = 4    # treatment dim
HD = 64   # hidden dim
IN_DIM = FD + TD
STRIDE = 9               # coarse-grid stride for the non-final sweeps
NC = (T - 1) // STRIDE + 1   # 12 coarse samples (hits both endpoints)
NSWEEP = 2
PLAN = NSWEEP  # kept for test.py / prof.py compatibility
N_CORES = 8

_F32 = mybir.dt.float32
_BF = mybir.dt.bfloat16

# d ([102, DW] bf16) column layout:
#   STC | ST | W1 | B1 | B2 | W2 | W3 || ATC | ATA | FAB
_C_STC = 0                  # [36, NC] coarse init state (x0 bcast + tr[::9]^T)
_C_ST = _C_STC + NC         # [36, T]  full state (rows 32:36 = tr^T from host)
_C_W1 = _C_ST + T           # [36, 64]
_C_B1 = _C_W1 + HD          # fp32 [64,1] as 2 bf16 cols (byte off 4-aligned)
_C_B2 = _C_B1 + 2
_C_W2 = _C_B2 + 2           # [64, 64]
_C_W3 = _C_W2 + HD          # [64, 32]
_C1 = _C_W3 + FD            # end of DMA1 (rows 0:64)
_C_ATC = _C1                # [NC+2, T]: Ar^T | ones | ts-t0  (coarse->fine)
_C_ATA = _C_ATC + T         # [T+2, T]:  A^T  | ones | ts-t0  (fine->fine)
_C_FABC = _C_ATA + T        # [NC+2, 32]: rows NC:NC+2 = [x0; b3]; 0:NC live F
_C_FAB = _C_FABC + FD       # [T+2, 32]:  rows T:T+2   = [x0; b3]; 0:T  live F
DW = _C_FAB + FD
DP = T + 2                  # partition count


class _LeanBlock(bass.BassBlock):
    """Block whose exit skips the all-engine EVSEM butterfly: engines just
    drain and end.  Output integrity is guaranteed by the sync stream's
    final wait on the output-DMA semaphore; semaphores are re-cleared by
    the preamble on every execution."""

    def __exit__(self, exc_type, exc_val, exc_tb):
        if exc_type is not None:
            return
        for engine, last_body in self.last_body.items():
            with self.bass.body(
                last_body, parent=self.bass.cur_bb, allow_existing_parent=True
            ):
                engine.br(self.end_bb)
        self.bass.switch_bb(self.end_bb)
        sync_type = self.bass.sync.engine
        for eng_type, eng in self.bass.engines.items():
            if eng_type != sync_type:
                continue
            d = mybir.InstDrain(
                name=self.bass.get_next_instruction_name(),
                ins=[],
                outs=[],
                bass_is_fusable=False,
            )
            d.engine = eng_type
            eng.add_instruction(d)


def _build_nc(nsweep=NSWEEP):
    nc = bass.Bass(trn_type="TRN2", monotonic_sem_count=0, enable_partition_id=False)
    d = nc.dram_tensor("d", [DP, DW], _BF, kind="ExternalInput")
    xt = nc.dram_tensor("xt", [FD, T], _F32, kind="ExternalOutput")

    tanh = mybir.ActivationFunctionType.Tanh

    with ExitStack() as ctx:
        sb = lambda nm, shape, dt: ctx.enter_context(nc.sbuf_tensor(nm, shape, dt))
        ps = lambda nm, shape: ctx.enter_context(nc.psum_tensor(nm, shape, _F32))
        sem = lambda nm: ctx.enter_context(nc.semaphore(nm))
        t = sb("t_d", [DP, DW], _BF)
        h1 = sb("t_h1", [HD, T], _BF)
        h2 = sb("t_h2", [HD, T], _BF)
        warm = sb("t_warm", [HD, 1], _F32)
        xt_sb = sb("t_xt", [FD, T], _F32)
        p1 = ps("t_p1", [HD, T])
        p2 = ps("t_p2", [HD, T])
        pf = ps("t_pf", [T, FD])
        px = ps("t_px", [FD, T])
        sem_d1 = sem("sem_d1")
        sem_d2 = sem("sem_d2")
        pe_sem = sem("sem_pe")
        act_sem = sem("sem_act")
        dve_sem = sem("sem_dve")

        st_c = t[0:IN_DIM, _C_STC:_C_STC + NC]
        st = t[0:IN_DIM, _C_ST:_C_ST + T]
        w1 = t[0:IN_DIM, _C_W1:_C_W1 + HD]
        w2 = t[0:HD, _C_W2:_C_W2 + HD]
        w3 = t[0:HD, _C_W3:_C_W3 + FD]
        b1 = t[0:HD, _C_B1:_C_B1 + 2].bitcast(_F32)
        b2 = t[0:HD, _C_B2:_C_B2 + 2].bitcast(_F32)
        atc = t[0:2 + NC, _C_ATC:_C_ATC + T]
        ata = t[0:DP, _C_ATA:_C_ATA + T]
        fab_c = t[0:2 + NC, _C_FABC:_C_FABC + FD]
        fab_f = t[0:DP, _C_FAB:_C_FAB + FD]

        block = ctx.enter_context(_LeanBlock(nc, 'blk'))

        # per-sweep sample count / operand views (last sweep = full grid)
        def sw(j):
            full = j == nsweep - 1
            n = T if full else NC
            return dict(
                n=n,
                stv=st if full else st_c,
                fab_w=t[0:n, (_C_FAB if full else _C_FABC):(_C_FAB if full else _C_FABC) + FD],
                mm4_l=fab_f if full else fab_c,
                mm4_r=ata if full else atc,
            )

        # semaphore plan (DMA sems inc by 16 at transfer completion):
        #   sem_d1: DMA1 (state+weights+biases) = 16; output DMA -> 32
        #   sem_d2: DMA2 (quadrature + fab consts) = 16
        #   pe_sem:  4 matmuls/sweep -> 4j+k after k-th matmul of sweep j
        #   act_sem: 2 tanhs/sweep   -> 2j+k
        #   dve_sem: 2 copies/sweep (fab, state; last sweep: fab, fp32 out)
        #   (output DMA completion rides sem_d1 -> 32)

        @block.gpsimd
        def _(gpsimd):
            nc.gpsimd.dma_start(t[0:DP, _C1:DW], d[0:DP, _C1:DW]).then_inc(sem_d2, 16)

        @block.sync
        def _(sync):
            sync.dma_start(t[0:HD, 0:_C1], d[0:HD, 0:_C1]).then_inc(sem_d1, 16)
            sync.wait_ge(pe_sem, 4 * nsweep)
            sync.dma_start(xt[:, :], xt_sb[:, :]).then_inc(sem_d1, 16)
            sync.wait_ge(sem_d1, 32)

        @block.tensor
        def _(tensor):
            tensor.wait_ge(sem_d1, 16)
            for j in range(nsweep):
                s = sw(j)
                n = s["n"]
                if j > 0:
                    tensor.wait_ge(dve_sem, 2 * j)
                nc.tensor.matmul(p1[:, 0:n], w1, s["stv"], start=True, stop=True).then_inc(pe_sem, 1)
                tensor.wait_ge(act_sem, 2 * j + 1)
                nc.tensor.matmul(p2[:, 0:n], w2, h1[:, 0:n], start=True, stop=True).then_inc(pe_sem, 1)
                tensor.wait_ge(act_sem, 2 * j + 2)
                nc.tensor.matmul(pf[0:n, :], h2[:, 0:n], w3, start=True, stop=True).then_inc(pe_sem, 1)
                tensor.wait_ge(dve_sem, 2 * j + 1)
                if j == 0:
                    tensor.wait_ge(sem_d2, 16)
                nc.tensor.matmul(px[:, :], s["mm4_l"], s["mm4_r"], start=True, stop=True).then_inc(pe_sem, 1)

        @block.scalar
        def _(scalar):
            # dep-free warm-up: zero a scratch tile, tanh it -> loads the
            # Tanh table while the input DMAs are still in flight.
            nc.scalar.mul(warm[:, :], warm[:, :], 0.0)
            nc.scalar.activation(warm[:, :], warm[:, :], tanh)
            for j in range(nsweep):
                n = sw(j)["n"]
                scalar.wait_ge(pe_sem, 4 * j + 1)
                nc.scalar.activation(h1[:, 0:n], p1[:, 0:n], tanh, bias=b1).then_inc(act_sem, 1)
                scalar.wait_ge(pe_sem, 4 * j + 2)
                nc.scalar.activation(h2[:, 0:n], p2[:, 0:n], tanh, bias=b2).then_inc(act_sem, 1)

        @block.vector
        def _(vector):
            for j in range(nsweep):
                s = sw(j)
                n = s["n"]
                vector.wait_ge(pe_sem, 4 * j + 3)
                nc.vector.tensor_copy(s["fab_w"], pf[0:n, :]).then_inc(dve_sem, 1)
                vector.wait_ge(pe_sem, 4 * j + 4)
                if j < nsweep - 1:
                    nc.vector.tensor_copy(st[0:FD, :], px[:, :]).then_inc(dve_sem, 1)
                else:
                    nc.vector.tensor_copy(xt_sb[:, :], px[:, :]).then_inc(dve_sem, 1)

    return nc


_NC_CACHE = {}


def _get_nc(nsweep=NSWEEP):
    nsweep = int(nsweep)
    if nsweep not in _NC_CACHE:
        _NC_CACHE[nsweep] = _build_nc(nsweep)
    return _NC_CACHE[nsweep]


def _host_prep(x0, treatments, ts, W1, b1, W2, b2, W3, b3):
    ts64 = ts.astype(np.float64)
    x032 = np.ascontiguousarray(x0.reshape(FD), dtype=np.float32)

    # cumulative trapezoid quadrature matrix A [T,T]:
    # (A @ F)[t] ~= \int_{t_0}^{t_t} f dt  for F sampled at the grid times.
    h = np.diff(ts64)
    A = np.zeros((T, T), np.float64)
    for k in range(T - 1):
        A[k + 1] = A[k]
        A[k + 1, k] += h[k] / 2
        A[k + 1, k + 1] += h[k] / 2

    # rectangular coarse->fine matrix Ar [T, NC]: cumulative integral of the
    # piecewise-linear interpolant on the coarse grid, evaluated at fine ts.
    idx = np.arange(0, T, STRIDE)
    assert idx[-1] == T - 1 and len(idx) == NC
    tsc = ts64[idx]
    Ar = np.zeros((T, NC), np.float64)
    for ti in range(T):
        tt = ts64[ti]
        for k in range(NC - 1):
            t0c, t1c = tsc[k], tsc[k + 1]
            if tt <= t0c:
                break
            w = min(tt, t1c) - t0c
            frac = w / (t1c - t0c)
            Ar[ti, k] += w * (1 - frac / 2)
            Ar[ti, k + 1] += w * frac / 2

    D = np.zeros((DP, DW), dtype=ml_dtypes.bfloat16)
    D[0:FD, _C_STC:_C_STC + NC] = x032[:, None]
    D[FD:IN_DIM, _C_STC:_C_STC + NC] = treatments[idx].T
    D[0:FD, _C_ST:_C_ST + T] = x032[:, None]   # init for the nsweep=1 path;
    D[FD:IN_DIM, _C_ST:_C_ST + T] = treatments.T  # overwritten by sweep-0 copy otherwise
    D[0:IN_DIM, _C_W1:_C_W1 + HD] = W1
    D[0:HD, _C_W2:_C_W2 + HD] = W2
    D[0:HD, _C_W3:_C_W3 + FD] = W3
    for off, foff, M in ((_C_ATC, _C_FABC, Ar), (_C_ATA, _C_FAB, A)):
        ncol = M.shape[1]
        D[0:ncol, off:off + T] = M.T
        D[ncol, off:off + T] = 1.0
        D[ncol + 1, off:off + T] = ts64 - ts64[0]
        D[ncol, foff:foff + FD] = x032
        D[ncol + 1, foff:foff + FD] = b3
    # biases stay exact fp32, stored as raw bf16-bit pairs
    u16 = D.view(np.uint16)
    u16[0:HD, _C_B1:_C_B1 + 2] = np.ascontiguousarray(
        b1.reshape(HD, 1), dtype=np.float32).view(np.uint16)
    u16[0:HD, _C_B2:_C_B2 + 2] = np.ascontiguousarray(
        b2.reshape(HD, 1), dtype=np.float32).view(np.uint16)
    return {"d": D}


def kernel(x0, treatments, ts, W1, b1, W2, b2, W3, b3, _results=None, _plan=NSWEEP):
    in_map = _host_prep(x0, treatments, ts, W1, b1, W2, b2, W3, b3)
    nc = _get_nc(_plan)
    res = bass_utils.run_bass_kernel_spmd(
        nc, [in_map] * N_CORES, core_ids=list(range(N_CORES))
    )
    if _results is not None:
        _results.append(res)
    xt = res.results[0]["xt"]  # [FD, T]
    out = xt.T.reshape(T, 1, FD)
    return np.ascontiguousarray(out, dtype=np.float32)
